# revision 11
# baseline (speedup 1.0000x reference)
import numpy as np

# nn_NearestNeighbours: batch [8,512,512] f32, emb [50000,512] f32,
# output argmin cosine-distance indices [8,512] int32.
#
# Strategy: vocab-sharded fp8 DoubleRow screen GEMM over 8 cores
# (6144 vocab cols per core, 49152 total; the 848-col tail is scored
# exactly on the host). Per 128-row m-tile each core evicts raw-dot
# scores PSUM->SBUF as contiguous f16 (split between ACT and DVE),
# folds 6144 -> 3072 -> 1536 with f16 tensor-max (2x DVE mode), and
# DMAs the folded cell table to HBM. The host finds cells within a
# screen margin of the row-global best, expands each cell to its 4
# source columns, and exact-rescores candidates in f64 on normalized
# embeddings (plus the tail block) to pick the final argmax.
B, S, E, V = 8, 512, 512, 50000
R = B * S              # 4096 token rows
NC = 8                 # cores
VS = 6144              # vocab cols per core on device
VTAIL = NC * VS        # 49152; [VTAIL, V) scored on host
KT = E // 128          # 4 k-subtiles
MT = R // 128          # 32 m-tiles
CH = 2048              # psum chunk width (3 chunks per m-tile)
DS = 640               # trailing cols of EACH chunk evicted by DVE (rest ACT)
FW = 1536              # folded cell-table width per m-tile
MARGIN = 16.0          # screen-score pruning margin (max observed gap 12.3)

_CACHE = {}


def _build():
    import concourse.bacc as bacc
    import concourse.mybir as mybir
    from concourse.tile import TileContext

    dtf = mybir.dt.float32
    dt8 = mybir.dt.float8e4
    dth = mybir.dt.float16
    DR = mybir.MatmulPerfMode.DoubleRow
    Copy = mybir.ActivationFunctionType.Copy

    nc = bacc.Bacc("TRN2", target_bir_lowering=False, debug=False)
    bT_ap = nc.dram_tensor("bT", [E, R], dt8, kind="ExternalInput").ap()
    embT_ap = nc.dram_tensor("embT", [E, VS], dt8, kind="ExternalInput").ap()
    outF_ap = nc.dram_tensor("outF", [R, FW], dth, kind="ExternalOutput").ap()

    with TileContext(nc) as tc:
        with tc.sbuf_pool(name="emb", bufs=1) as embp, \
             tc.sbuf_pool(name="bt", bufs=2) as btp, \
             tc.sbuf_pool(name="wk", bufs=1) as wkp, \
             tc.psum_pool(name="ps", bufs=2) as ps:
            engs = [nc.sync, nc.scalar, nc.gpsimd]
            gt = btp.tile([128, KT, 512], dt8)
            for k in range(KT):
                engs[k % 3].dma_start(gt[:, k:k + 1, :],
                                      bT_ap[128 * k:128 * (k + 1), 0:512])
            emb8 = embp.tile([128, KT, VS], dt8, name="emb8")
            # first 1024 cols in small pieces spread over 4 trigger queues so
            # the first matmuls start ASAP; the rest in 1024-wide pieces
            for off in range(0, 1024, 512):
                for k in range(KT):
                    engs[k % 3].dma_start(
                        emb8[:, k:k + 1, off:off + 512],
                        embT_ap[128 * k:128 * (k + 1), off:off + 512],
                    )
            for off in range(1024, VS, 1024):
                for k in range(KT):
                    engs[(off // 1024 + k) % 3].dma_start(
                        emb8[:, k:k + 1, off:off + 1024],
                        embT_ap[128 * k:128 * (k + 1), off:off + 1024],
                    )
            # h triple-buffered: evictions of m-tile m+2 must not wait on the
            # fold-tree reads of m-tile m
            h = wkp.tile([128, 3, VS], dth, name="h")
            t1 = wkp.tile([128, 2, VS // 2], dth, name="t1")
            fc = wkp.tile([128, 2, FW], dth, name="fc")

            for g in range(MT // 4):
                cur = gt
                if g + 1 < MT // 4:
                    gt = btp.tile([128, KT, 512], dt8)
                    for k in range(KT):
                        nc.sync.dma_start(
                            gt[:, k:k + 1, :],
                            bT_ap[128 * k:128 * (k + 1), 512 * (g + 1):512 * (g + 2)],
                        )
                for mm in range(4):
                    m = g * 4 + mm
                    hf = m % 3
                    h2 = m % 2
                    for c in range(3):
                        pt = ps.tile([128, CH], dtf)
                        for p in range(2):
                            for k in range(4):
                                nc.tensor.matmul(
                                    pt[:, 512 * k:512 * (k + 1)],
                                    cur[:, 2 * p:2 * p + 2, 128 * mm:128 * mm + 128],
                                    emb8[:, 2 * p:2 * p + 2,
                                         CH * c + 512 * k:CH * c + 512 * (k + 1)],
                                    start=(p == 0),
                                    stop=(p == 1),
                                    perf_mode=DR,
                                )
                        nc.scalar.activation(
                            h[:, hf, CH * c:CH * (c + 1) - DS],
                            pt[:, 0:CH - DS], Copy)
                        nc.vector.tensor_scalar_add(
                            h[:, hf, CH * (c + 1) - DS:CH * (c + 1)],
                            pt[:, CH - DS:CH], 0.0)
                    half = VS // 2
                    nc.vector.tensor_max(
                        t1[:, h2, :], h[:, hf, 0:half], h[:, hf, half:VS])
                    nc.vector.tensor_max(
                        fc[:, h2, :], t1[:, h2, 0:FW], t1[:, h2, FW:half])
                    nc.gpsimd.dma_start(
                        outF_ap[128 * m:128 * (m + 1), :], fc[:, h2, :])
    nc.compile()
    return nc


def _run(batch: np.ndarray, emb: np.ndarray, trace: bool = False):
    import ml_dtypes
    from concourse import bass_utils

    if "nc" not in _CACHE:
        _CACHE["nc"] = _build()
    nc = _CACHE["nc"]
    f8 = ml_dtypes.float8_e4m3

    b = np.ascontiguousarray(batch.reshape(R, E).astype(np.float32))
    bT8 = np.ascontiguousarray(b.T).astype(f8)
    embT8 = emb[:VTAIL].T.astype(f8)
    in_maps = []
    for c in range(NC):
        in_maps.append({
            "bT": bT8,
            "embT": np.ascontiguousarray(embT8[:, c * VS:(c + 1) * VS]),
        })

    res = bass_utils.run_bass_kernel_spmd(
        nc, in_maps, core_ids=list(range(NC)), trace=trace
    )

    # [R, NC, FW] screen cell table (cell j = max of 4 source columns)
    Fall = np.stack(
        [np.asarray(res.results[c]["outF"]) for c in range(NC)], axis=1
    ).astype(np.float32)
    gbest = Fall.max(axis=(1, 2))
    rows, cores, cells = np.nonzero(Fall >= (gbest - MARGIN)[:, None, None])

    # expand cells to their 4 source columns -> global vocab candidates
    cand = (cores * VS)[:, None] + cells[:, None] + \
        np.array([0, FW, 2 * FW, 3 * FW])[None, :]
    cand = cand.reshape(-1)
    crow = np.repeat(rows, 4)

    # exact rescore in f64 on normalized embeddings
    emb64 = emb.astype(np.float64)
    inv = 1.0 / np.sqrt((emb64 * emb64).sum(axis=1) + 1e-12)
    b64 = b.astype(np.float64)
    s = np.einsum("ij,ij->i", emb64[cand] * inv[cand][:, None], b64[crow])

    best_s = np.full(R, -np.inf)
    best_i = np.zeros(R, np.int64)
    order = np.argsort(crow, kind="stable")
    crow_s, cand_s, s_s = crow[order], cand[order], s[order]
    np.maximum.at(best_s, crow_s, s_s)
    hit = s_s == best_s[crow_s]
    best_i[crow_s[hit]] = cand_s[hit]
    # (ties: last writer wins; resolved below against exact tail anyway)

    # exact tail block [VTAIL, V)
    tail = (emb64[VTAIL:] * inv[VTAIL:, None]) @ b64.T   # [848, R]
    t_best = tail.max(axis=0)
    t_idx = VTAIL + tail.argmax(axis=0)
    use_tail = t_best > best_s
    best_i[use_tail] = t_idx[use_tail]

    return best_i.astype(np.int32).reshape(B, S), res


def kernel(batch: np.ndarray, emb: np.ndarray) -> np.ndarray:
    out, _ = _run(batch, emb, trace=False)
    return out


# revision 12
# speedup vs baseline: 1.0065x; 1.0065x over previous
import numpy as np

# nn_NearestNeighbours: batch [8,512,512] f32, emb [50000,512] f32,
# output argmin cosine-distance indices [8,512] int32.
#
# Strategy: vocab-sharded fp8 DoubleRow screen GEMM over 8 cores
# (6144 vocab cols per core, 49152 total; the 848-col tail is scored
# exactly on the host). Per 128-row m-tile each core evicts raw-dot
# scores PSUM->SBUF as contiguous f16 (split between ACT and DVE),
# folds 6144 -> 3072 -> 1536 with f16 tensor-max (2x DVE mode), and
# DMAs the folded cell table to HBM. The host finds cells within a
# screen margin of the row-global best, expands each cell to its 4
# source columns, and exact-rescores candidates in f64 on normalized
# embeddings (plus the tail block) to pick the final argmax.
B, S, E, V = 8, 512, 512, 50000
R = B * S              # 4096 token rows
NC = 8                 # cores
VS = 6144              # vocab cols per core on device
VTAIL = NC * VS        # 49152; [VTAIL, V) scored on host
KT = E // 128          # 4 k-subtiles
MT = R // 128          # 32 m-tiles
CH = 2048              # psum chunk width (3 chunks per m-tile)
DS = 640               # trailing cols of EACH chunk evicted by DVE (rest ACT)
FW = 1536              # folded cell-table width per m-tile
MARGIN = 16.0          # screen-score pruning margin (max observed gap 12.3)

_CACHE = {}


def _build():
    import concourse.bacc as bacc
    import concourse.mybir as mybir
    from concourse.tile import TileContext

    dtf = mybir.dt.float32
    dt8 = mybir.dt.float8e4
    dth = mybir.dt.float16
    DR = mybir.MatmulPerfMode.DoubleRow
    Copy = mybir.ActivationFunctionType.Copy

    nc = bacc.Bacc("TRN2", target_bir_lowering=False, debug=False)
    bT_ap = nc.dram_tensor("bT", [E, R], dt8, kind="ExternalInput").ap()
    embT_ap = nc.dram_tensor("embT", [E, VS], dt8, kind="ExternalInput").ap()
    outF_ap = nc.dram_tensor("outF", [R, FW], dth, kind="ExternalOutput").ap()

    with TileContext(nc) as tc:
        with tc.sbuf_pool(name="emb", bufs=1) as embp, \
             tc.sbuf_pool(name="bt", bufs=2) as btp, \
             tc.sbuf_pool(name="wk", bufs=1) as wkp, \
             tc.psum_pool(name="ps", bufs=2) as ps:
            engs = [nc.sync, nc.scalar, nc.gpsimd]
            gt = btp.tile([128, KT, 512], dt8)
            for k in range(KT):
                engs[k % 3].dma_start(gt[:, k:k + 1, :],
                                      bT_ap[128 * k:128 * (k + 1), 0:512])
            emb8 = embp.tile([128, KT, VS], dt8, name="emb8")
            # first 1024 cols in small pieces spread over 4 trigger queues so
            # the first matmuls start ASAP; the rest in 1024-wide pieces
            for off in range(0, 1024, 512):
                for k in range(KT):
                    engs[k % 3].dma_start(
                        emb8[:, k:k + 1, off:off + 512],
                        embT_ap[128 * k:128 * (k + 1), off:off + 512],
                    )
            for off in range(1024, VS, 1024):
                for k in range(KT):
                    engs[(off // 1024 + k) % 3].dma_start(
                        emb8[:, k:k + 1, off:off + 1024],
                        embT_ap[128 * k:128 * (k + 1), off:off + 1024],
                    )
            # h triple-buffered: evictions of m-tile m+2 must not wait on the
            # fold-tree reads of m-tile m
            h = wkp.tile([128, 3, VS], dth, name="h")
            t1 = wkp.tile([128, 2, VS // 2], dth, name="t1")
            fc = wkp.tile([128, 2, FW], dth, name="fc")

            for g in range(MT // 4):
                cur = gt
                if g + 1 < MT // 4:
                    gt = btp.tile([128, KT, 512], dt8)
                    for k in range(KT):
                        nc.sync.dma_start(
                            gt[:, k:k + 1, :],
                            bT_ap[128 * k:128 * (k + 1), 512 * (g + 1):512 * (g + 2)],
                        )
                for mm in range(4):
                    m = g * 4 + mm
                    hf = m % 3
                    half = VS // 2

                    # fold tree of m-tile pm, emitted one m-tile behind and
                    # interleaved between evicts so the in-order DVE queue
                    # fills its PE-wait gaps instead of delaying PSUM release
                    def tree_a(pm):
                        nc.vector.tensor_max(
                            t1[:, pm % 2, :],
                            h[:, pm % 3, 0:half], h[:, pm % 3, half:VS])

                    def tree_b(pm):
                        nc.vector.tensor_max(
                            fc[:, pm % 2, :],
                            t1[:, pm % 2, 0:FW], t1[:, pm % 2, FW:half])
                        nc.gpsimd.dma_start(
                            outF_ap[128 * pm:128 * (pm + 1), :],
                            fc[:, pm % 2, :])

                    for c in range(3):
                        pt = ps.tile([128, CH], dtf)
                        for p in range(2):
                            for k in range(4):
                                nc.tensor.matmul(
                                    pt[:, 512 * k:512 * (k + 1)],
                                    cur[:, 2 * p:2 * p + 2, 128 * mm:128 * mm + 128],
                                    emb8[:, 2 * p:2 * p + 2,
                                         CH * c + 512 * k:CH * c + 512 * (k + 1)],
                                    start=(p == 0),
                                    stop=(p == 1),
                                    perf_mode=DR,
                                )
                        nc.scalar.activation(
                            h[:, hf, CH * c:CH * (c + 1) - DS],
                            pt[:, 0:CH - DS], Copy)
                        nc.vector.tensor_scalar_add(
                            h[:, hf, CH * (c + 1) - DS:CH * (c + 1)],
                            pt[:, CH - DS:CH], 0.0)
                        if m >= 1 and c == 0:
                            tree_a(m - 1)
                        elif m >= 1 and c == 1:
                            tree_b(m - 1)
            tree_a(MT - 1)
            tree_b(MT - 1)
    nc.compile()
    return nc


def _run(batch: np.ndarray, emb: np.ndarray, trace: bool = False):
    import ml_dtypes
    from concourse import bass_utils

    if "nc" not in _CACHE:
        _CACHE["nc"] = _build()
    nc = _CACHE["nc"]
    f8 = ml_dtypes.float8_e4m3

    b = np.ascontiguousarray(batch.reshape(R, E).astype(np.float32))
    bT8 = np.ascontiguousarray(b.T).astype(f8)
    embT8 = emb[:VTAIL].T.astype(f8)
    in_maps = []
    for c in range(NC):
        in_maps.append({
            "bT": bT8,
            "embT": np.ascontiguousarray(embT8[:, c * VS:(c + 1) * VS]),
        })

    res = bass_utils.run_bass_kernel_spmd(
        nc, in_maps, core_ids=list(range(NC)), trace=trace
    )

    # [R, NC, FW] screen cell table (cell j = max of 4 source columns)
    Fall = np.stack(
        [np.asarray(res.results[c]["outF"]) for c in range(NC)], axis=1
    ).astype(np.float32)
    gbest = Fall.max(axis=(1, 2))
    rows, cores, cells = np.nonzero(Fall >= (gbest - MARGIN)[:, None, None])

    # expand cells to their 4 source columns -> global vocab candidates
    cand = (cores * VS)[:, None] + cells[:, None] + \
        np.array([0, FW, 2 * FW, 3 * FW])[None, :]
    cand = cand.reshape(-1)
    crow = np.repeat(rows, 4)

    # exact rescore in f64 on normalized embeddings
    emb64 = emb.astype(np.float64)
    inv = 1.0 / np.sqrt((emb64 * emb64).sum(axis=1) + 1e-12)
    b64 = b.astype(np.float64)
    s = np.einsum("ij,ij->i", emb64[cand] * inv[cand][:, None], b64[crow])

    best_s = np.full(R, -np.inf)
    best_i = np.zeros(R, np.int64)
    order = np.argsort(crow, kind="stable")
    crow_s, cand_s, s_s = crow[order], cand[order], s[order]
    np.maximum.at(best_s, crow_s, s_s)
    hit = s_s == best_s[crow_s]
    best_i[crow_s[hit]] = cand_s[hit]
    # (ties: last writer wins; resolved below against exact tail anyway)

    # exact tail block [VTAIL, V)
    tail = (emb64[VTAIL:] * inv[VTAIL:, None]) @ b64.T   # [848, R]
    t_best = tail.max(axis=0)
    t_idx = VTAIL + tail.argmax(axis=0)
    use_tail = t_best > best_s
    best_i[use_tail] = t_idx[use_tail]

    return best_i.astype(np.int32).reshape(B, S), res


def kernel(batch: np.ndarray, emb: np.ndarray) -> np.ndarray:
    out, _ = _run(batch, emb, trace=False)
    return out


# revision 13
# speedup vs baseline: 1.1762x; 1.1686x over previous
import numpy as np

# nn_NearestNeighbours: batch [8,512,512] f32, emb [50000,512] f32,
# output argmin cosine-distance indices [8,512] int32.
#
# Strategy: vocab-sharded fp8 DoubleRow screen GEMM over 8 cores
# (6144 vocab cols per core, 49152 total; the 848-col tail is scored
# exactly on the host). Per 128-row m-tile each core evicts raw-dot
# scores PSUM->SBUF as contiguous f16 (chunks 0-3 on ACT, 4-5 on DVE),
# folds 6144 -> 3072 -> 1536 with f16 tensor-max (2x DVE mode, emitted
# one m-tile deferred so the in-order DVE queue never delays PSUM
# release), and DMAs the folded cell table to HBM. The host finds
# cells within a screen margin of the row-global best, expands each
# cell to its 4 source columns, and exact-rescores candidates in f64
# on normalized embeddings (plus the tail block) to pick the argmax.
B, S, E, V = 8, 512, 512, 50000
R = B * S              # 4096 token rows
NC = 8                 # cores
VS = 6144              # vocab cols per core on device
VTAIL = NC * VS        # 49152; [VTAIL, V) scored on host
KT = E // 128          # 4 k-subtiles
MT = R // 128          # 32 m-tiles
CHW = 1024             # psum chunk width (6 chunks per m-tile)
NCH = VS // CHW
NACT = 4               # leading chunks evicted by ACT; rest by DVE
FW = 1536              # folded cell-table width per m-tile
MARGIN = 16.0          # screen-score pruning margin (max observed gap 12.3)

_CACHE = {}


def _build():
    import concourse.bacc as bacc
    import concourse.mybir as mybir
    from concourse.tile import TileContext

    dtf = mybir.dt.float32
    dt8 = mybir.dt.float8e4
    dth = mybir.dt.float16
    DR = mybir.MatmulPerfMode.DoubleRow
    Copy = mybir.ActivationFunctionType.Copy

    nc = bacc.Bacc("TRN2", target_bir_lowering=False, debug=False)
    bT_ap = nc.dram_tensor("bT", [E, R], dt8, kind="ExternalInput").ap()
    embT_ap = nc.dram_tensor("embT", [E, VS], dt8, kind="ExternalInput").ap()
    outF_ap = nc.dram_tensor("outF", [R, FW], dth, kind="ExternalOutput").ap()

    with TileContext(nc) as tc:
        with tc.sbuf_pool(name="emb", bufs=1) as embp, \
             tc.sbuf_pool(name="bt", bufs=2) as btp, \
             tc.sbuf_pool(name="wk", bufs=1) as wkp, \
             tc.psum_pool(name="ps", bufs=3) as ps:
            engs = [nc.sync, nc.scalar, nc.gpsimd]
            gt = btp.tile([128, KT, 512], dt8)
            for k in range(KT):
                engs[k % 3].dma_start(gt[:, k:k + 1, :],
                                      bT_ap[128 * k:128 * (k + 1), 0:512])
            emb8 = embp.tile([128, KT, VS], dt8, name="emb8")
            # first cols in small pieces over 3 trigger queues so the first
            # matmuls start ASAP; the rest in 1024-wide pieces
            for off in range(0, 512, 256):
                for k in range(KT):
                    engs[k % 3].dma_start(
                        emb8[:, k:k + 1, off:off + 256],
                        embT_ap[128 * k:128 * (k + 1), off:off + 256])
            for k in range(KT):
                engs[k % 3].dma_start(
                    emb8[:, k:k + 1, 512:1024],
                    embT_ap[128 * k:128 * (k + 1), 512:1024])
            for off in range(1024, VS, 1024):
                for k in range(KT):
                    engs[(off // 1024 + k) % 3].dma_start(
                        emb8[:, k:k + 1, off:off + 1024],
                        embT_ap[128 * k:128 * (k + 1), off:off + 1024])
            # h triple-buffered: the one-m-tile-deferred tree reads h[m-1]
            # while evictions of m and m+1 write the other two slots
            h = wkp.tile([128, 3, VS], dth, name="h")
            t1 = wkp.tile([128, 2, VS // 2], dth, name="t1")
            fc = wkp.tile([128, 2, FW], dth, name="fc")
            half = VS // 2

            def tree_a(pm):
                nc.vector.tensor_max(
                    t1[:, pm % 2, :],
                    h[:, pm % 3, 0:half], h[:, pm % 3, half:VS])

            def tree_b(pm):
                nc.vector.tensor_max(
                    fc[:, pm % 2, :], t1[:, pm % 2, 0:FW], t1[:, pm % 2, FW:half])
                nc.sync.dma_start(
                    outF_ap[128 * pm:128 * (pm + 1), :], fc[:, pm % 2, :])

            for g in range(MT // 4):
                cur = gt
                if g + 1 < MT // 4:
                    gt = btp.tile([128, KT, 512], dt8)
                    for k in range(KT):
                        nc.sync.dma_start(
                            gt[:, k:k + 1, :],
                            bT_ap[128 * k:128 * (k + 1), 512 * (g + 1):512 * (g + 2)])
                for mm in range(4):
                    m = g * 4 + mm
                    hf = m % 3
                    for c in range(NCH):
                        pt = ps.tile([128, CHW], dtf)
                        for p in range(2):
                            for k in range(CHW // 512):
                                nc.tensor.matmul(
                                    pt[:, 512 * k:512 * (k + 1)],
                                    cur[:, 2 * p:2 * p + 2, 128 * mm:128 * mm + 128],
                                    emb8[:, 2 * p:2 * p + 2,
                                         CHW * c + 512 * k:CHW * c + 512 * (k + 1)],
                                    start=(p == 0), stop=(p == 1), perf_mode=DR)
                        if c < NACT:
                            nc.scalar.activation(
                                h[:, hf, CHW * c:CHW * (c + 1)], pt[:], Copy)
                        else:
                            nc.vector.tensor_scalar_add(
                                h[:, hf, CHW * c:CHW * (c + 1)], pt[:], 0.0)
                        if m >= 1 and c == 0:
                            tree_a(m - 1)
                        elif m >= 1 and c == 1:
                            tree_b(m - 1)
            tree_a(MT - 1)
            tree_b(MT - 1)
    nc.compile()
    return nc


def _run(batch: np.ndarray, emb: np.ndarray, trace: bool = False):
    import ml_dtypes
    from concourse import bass_utils

    if "nc" not in _CACHE:
        _CACHE["nc"] = _build()
    nc = _CACHE["nc"]
    f8 = ml_dtypes.float8_e4m3

    b = np.ascontiguousarray(batch.reshape(R, E).astype(np.float32))
    bT8 = np.ascontiguousarray(b.T).astype(f8)
    embT8 = emb[:VTAIL].T.astype(f8)
    in_maps = []
    for c in range(NC):
        in_maps.append({
            "bT": bT8,
            "embT": np.ascontiguousarray(embT8[:, c * VS:(c + 1) * VS]),
        })

    res = bass_utils.run_bass_kernel_spmd(
        nc, in_maps, core_ids=list(range(NC)), trace=trace
    )

    # [R, NC, FW] screen cell table (cell j = max of 4 source columns)
    Fall = np.stack(
        [np.asarray(res.results[c]["outF"]) for c in range(NC)], axis=1
    ).astype(np.float32)
    gbest = Fall.max(axis=(1, 2))
    rows, cores, cells = np.nonzero(Fall >= (gbest - MARGIN)[:, None, None])

    # expand cells to their 4 source columns -> global vocab candidates
    cand = (cores * VS)[:, None] + cells[:, None] + \
        np.array([0, FW, 2 * FW, 3 * FW])[None, :]
    cand = cand.reshape(-1)
    crow = np.repeat(rows, 4)

    # exact rescore in f64 on normalized embeddings
    emb64 = emb.astype(np.float64)
    inv = 1.0 / np.sqrt((emb64 * emb64).sum(axis=1) + 1e-12)
    b64 = b.astype(np.float64)
    s = np.einsum("ij,ij->i", emb64[cand] * inv[cand][:, None], b64[crow])

    best_s = np.full(R, -np.inf)
    best_i = np.zeros(R, np.int64)
    np.maximum.at(best_s, crow, s)
    hit = s == best_s[crow]
    best_i[crow[hit]] = cand[hit]

    # exact tail block [VTAIL, V)
    tail = (emb64[VTAIL:] * inv[VTAIL:, None]) @ b64.T   # [848, R]
    t_best = tail.max(axis=0)
    t_idx = VTAIL + tail.argmax(axis=0)
    use_tail = t_best > best_s
    best_i[use_tail] = t_idx[use_tail]

    return best_i.astype(np.int32).reshape(B, S), res


def kernel(batch: np.ndarray, emb: np.ndarray) -> np.ndarray:
    out, _ = _run(batch, emb, trace=False)
    return out
